# revision 13
# baseline (speedup 1.0000x reference)
"""DETR scene-graph predicate head on 8 Trainium2 NeuronCores.

Math: logits[l,b,r,:] = concat(hs[l,b,q_sub], hs[l,b,q_obj]) @ W_pred.T + b_pred
where q_sub/q_obj are derived from (tgt_perm inverse, relationships,
src_indices) — pure integer index math, done on host.

Strategy (batch axis sharded 8 ways; L*B/8 = 192 (layer,image) blocks/core):
  - The host performs the gather itself (it owns the indices anyway) and
    ships the gathered pair representations pre-transposed: per block, four
    [128, 64] bf16 chunks c=(sub/obj, d-half) with d on partitions and
    relation r on columns. This removes the on-chip one-hot gather matmuls,
    the psum->bf16 cast traffic, and halves input DMA bytes vs shipping
    hs + one-hot selectors.
  - Kernel per group of G=8 blocks: 4 accumulating matmuls (lhsT = W chunk
    [128, 51] stationary, rhs streams 512 cols = 8 blocks x 64 relations)
    into a [51, 512] psum region. Groups are paired via tile_position
    (0,0)/(0,64): outputs land on psum partitions 0:51 / 64:115 of one
    bank, so the [128, 512] bf16 stores fan across all DMA engines.
  - Input DMA on the gpsimd (SWDGE) queue with staggered load sizes
    [2,2,4,4,4,4,2,2] groups: small loads at the ends shrink pipeline fill
    and drain; 8-16KB/partition descriptors keep ~370 GB/s sustained.
  - A short dense-matmul preamble plus keep-warm matmuls per load hold the
    PE clock (HAM) at 2.4 GHz through DMA-paced stretches.

hs and W_pred are bf16 on-chip (gather is exact; psum accumulates f32) and
the output travels bf16 over DMA before the host casts to f32, giving
~3e-3 relative error vs the f32 reference.
"""

import sys

import numpy as np

L, B, Q1, D = 6, 256, 101, 256
M, R, P = 64, 64, 51
NCORES = 8
BLOC = B // NCORES          # images per core
NB = L * BLOC               # (layer, image) blocks per core
G = 8                       # blocks per group (one 512-col psum region)
NG = NB // G                # groups per core
GCOLS = 4 * 512             # tile cols per group (4 chunks x 512)
SPLITS = [2, 2, 4, 4, 4, 4, 2, 1, 1]   # groups per DMA load
NPAIR = NG // 2

_CACHE = {}


def _build_program():
    import concourse.bacc as bacc
    import concourse.mybir as mybir
    import concourse.tile as tile
    from contextlib import ExitStack

    f32 = mybir.dt.float32
    bf16 = mybir.dt.bfloat16
    nc = bacc.Bacc("TRN2", target_bir_lowering=False, debug=False)

    pg = nc.dram_tensor("pg", [128, NG * GCOLS], bf16,
                        kind="ExternalInput").ap()
    wt = nc.dram_tensor("wt", [128, 4 * P], bf16, kind="ExternalInput").ap()
    bias = nc.dram_tensor("bias", [128, 1], f32, kind="ExternalInput").ap()
    out = nc.dram_tensor("out", [NPAIR, 128, 512], bf16,
                         kind="ExternalOutput").ap()

    with tile.TileContext(nc) as tc, ExitStack() as ctx:
        const = ctx.enter_context(tc.tile_pool(name="const", bufs=1))
        inp = ctx.enter_context(tc.tile_pool(name="inp", bufs=3))
        outp = ctx.enter_context(tc.tile_pool(name="outp", bufs=4))
        psW = ctx.enter_context(tc.tile_pool(name="psW", bufs=1, space="PSUM"))
        psO = ctx.enter_context(tc.tile_pool(name="psO", bufs=6, space="PSUM"))

        wt_t = const.tile([128, 4 * P], bf16)
        nc.sync.dma_start(out=wt_t[:], in_=wt[:])
        bias_t = const.tile([128, 1], f32)
        nc.sync.dma_start(out=bias_t[:], in_=bias[:])

        # HAM warm-up: dense N=512 matmuls push the PE clock 1.2 -> 2.4 GHz
        wu = const.tile([128, 512], bf16)
        nc.vector.memset(wu[:], 0.0)
        wps = psW.tile([128, 512], f32, tag="wu")
        for _ in range(12):
            nc.tensor.matmul(out=wps[:], lhsT=wu[:, 0:128], rhs=wu[:],
                             start=True, stop=True)

        goff = 0
        pair_idx = 0
        pending = []          # (tile, col offset) per loaded group, in order
        for sz in SPLITS:
            pg_t = inp.tile([128, 4 * GCOLS], bf16, tag="pg")
            nc.gpsimd.dma_start(out=pg_t[:, 0:sz * GCOLS],
                                in_=pg[:, goff * GCOLS:(goff + sz) * GCOLS])
            pending.extend((pg_t, s * GCOLS) for s in range(sz))
            goff += sz
            # keep-warm matmuls run while the PE waits on this load
            for _ in range(sz // 2):
                nc.tensor.matmul(out=wps[:], lhsT=wu[:, 0:128], rhs=wu[:],
                                 start=True, stop=True)
            while len(pending) >= 2:
                (t0, o0), (t1, o1) = pending[0], pending[1]
                del pending[:2]
                pO = psO.tile([128, 512], f32, tag="pO")
                o_t = outp.tile([128, 512], bf16, tag="o")
                for e, (te, oe) in enumerate(((t0, o0), (t1, o1))):
                    rows = slice(64 * e, 64 * e + P)
                    for c in range(4):
                        nc.tensor.matmul(out=pO[rows, :],
                                         lhsT=wt_t[:, c * P:(c + 1) * P],
                                         rhs=te[:, oe + c * 512:
                                                oe + (c + 1) * 512],
                                         start=(c == 0), stop=(c == 3),
                                         tile_position=(0, 64 * e))
                    if e == 0:
                        nc.vector.tensor_scalar_add(out=o_t[rows, :],
                                                    in0=pO[rows, :],
                                                    scalar1=bias_t[rows, :])
                    else:
                        nc.scalar.add(out=o_t[rows, :], in_=pO[rows, :],
                                      add=bias_t[rows, :])
                qeng = (nc.scalar, nc.sync)[pair_idx % 2]
                qeng.dma_start(out=out[pair_idx], in_=o_t[:])
                pair_idx += 1

    nc.compile()
    return nc


def _host_indices(src_indices, tgt_perm, relationships):
    """q_sub, q_obj: [L, B, R] int64 — matched query slot per relation."""
    src = np.asarray(src_indices, dtype=np.int64)
    tgt = np.asarray(tgt_perm, dtype=np.int64)
    rel = np.asarray(relationships, dtype=np.int64)

    # lookup[l, b, tgt[l, b, k]] = k
    lookup = np.empty((L, B, M), dtype=np.int64)
    li = np.arange(L)[:, None, None]
    bi = np.arange(B)[None, :, None]
    lookup[li, bi, tgt] = np.broadcast_to(np.arange(M), (L, B, M))

    sub_t = np.broadcast_to(rel[None, :, :, 0], (L, B, R))
    obj_t = np.broadcast_to(rel[None, :, :, 1], (L, B, R))
    pos_sub = np.take_along_axis(lookup, sub_t, axis=2)
    pos_obj = np.take_along_axis(lookup, obj_t, axis=2)
    q_sub = np.take_along_axis(src, pos_sub, axis=2)
    q_obj = np.take_along_axis(src, pos_obj, axis=2)
    return q_sub, q_obj


def _host_prepare(hs, src_indices, tgt_perm, relationships, W_pred, b_pred):
    """Build per-core input maps."""
    import ml_dtypes
    bf16 = ml_dtypes.bfloat16

    hs_bf = np.asarray(hs, dtype=np.float32).astype(bf16)
    W = np.asarray(W_pred, dtype=np.float32)
    b = np.asarray(b_pred, dtype=np.float32)

    q_sub, q_obj = _host_indices(src_indices, tgt_perm, relationships)
    q_cat = np.concatenate([q_sub, q_obj], axis=-1)          # [L, B, 2R]
    # gathered[l, b, j, :] = hs[l, b, q_cat[l, b, j], :]
    gathered = np.take_along_axis(hs_bf, q_cat[..., None], axis=2)

    # W chunks: wt[:, c*P + p] = W[p, c*128 + dd]
    wt_packed = np.ascontiguousarray(
        W.reshape(P, 4, 128).transpose(2, 1, 0).reshape(128, 4 * P)
    ).astype(bf16)
    # bias at partitions 0:51 (even groups) and 64:115 (odd groups)
    bias_col = np.zeros((128, 1), dtype=np.float32)
    bias_col[0:P, 0] = b
    bias_col[64:64 + P, 0] = b

    in_maps = []
    for core in range(NCORES):
        sl = slice(core * BLOC, (core + 1) * BLOC)
        # cols must be (group, c=(so, dh), j, r), d on partitions
        arr = gathered[:, sl].reshape(NB, 2, R, 2, 128)   # [nb, so, r, dh, dd]
        arr = arr.reshape(NG, G, 2, R, 2, 128)            # [g, j, so, r, dh,dd]
        pg_core = np.ascontiguousarray(
            arr.transpose(5, 0, 2, 4, 1, 3))              # [dd, g, so, dh, j,r]
        in_maps.append({
            "pg": pg_core.reshape(128, NG * GCOLS),
            "wt": wt_packed,
            "bias": bias_col,
        })
    return in_maps


def kernel(hs, src_indices, tgt_perm, relationships, W_pred, b_pred):
    if "concourse" not in sys.modules:
        try:
            import concourse  # noqa: F401
        except ImportError:
            sys.path.insert(0, "/opt/trn_rl_repo")
    from concourse import bass_utils

    in_maps = _host_prepare(hs, src_indices, tgt_perm, relationships,
                            W_pred, b_pred)
    if "nc" not in _CACHE:
        _CACHE["nc"] = _build_program()
    nc = _CACHE["nc"]

    res = bass_utils.run_bass_kernel_spmd(nc, in_maps, list(range(NCORES)))
    outs = []
    for core in range(NCORES):
        o = res.results[core]["out"]              # [NPAIR, 128, 512] bf16
        o = np.asarray(o, dtype=np.float32)
        t = o.reshape(NPAIR, 128, G, R)           # [pairi, row, j, r]
        comb = np.stack([t[:, 0:P], t[:, 64:64 + P]], axis=1)
        # comb: [pairi, e, p, j, r] -> [pairi, e, j, r, p]
        arr = comb.transpose(0, 1, 3, 4, 2).reshape(NB, R, P)
        outs.append(arr.reshape(L, BLOC, R, P))
    return np.concatenate(outs, axis=1)


# revision 14
# speedup vs baseline: 1.0683x; 1.0683x over previous
"""DETR scene-graph predicate head on 8 Trainium2 NeuronCores.

Math: logits[l,b,r,:] = concat(hs[l,b,q_sub], hs[l,b,q_obj]) @ W_pred.T + b_pred
where q_sub/q_obj are derived from (tgt_perm inverse, relationships,
src_indices) — pure integer index math, done on host.

Strategy (batch axis sharded 8 ways; L*B/8 = 192 (layer,image) blocks/core):
  - The host performs the gather itself (it owns the indices anyway) and
    ships the gathered pair representations pre-transposed: per block, four
    [128, 64] bf16 chunks c=(sub/obj, d-half) with d on partitions and
    relation r on columns. This removes the on-chip one-hot gather matmuls,
    the psum->bf16 cast traffic, and halves input DMA bytes vs shipping
    hs + one-hot selectors.
  - Kernel per group of G=8 blocks: 4 accumulating matmuls (lhsT = W chunk
    [128, 51] stationary, rhs streams 512 cols = 8 blocks x 64 relations)
    into a [51, 512] psum region. Groups are paired via tile_position
    (0,0)/(0,64): outputs land on psum partitions 0:51 / 64:115 of one
    bank, so the [128, 512] bf16 stores fan across all DMA engines.
  - Input DMA on the gpsimd (SWDGE) queue with staggered load sizes
    [2,2,4,4,4,4,2,2] groups: small loads at the ends shrink pipeline fill
    and drain; 8-16KB/partition descriptors keep ~370 GB/s sustained.
  - A short dense-matmul preamble plus keep-warm matmuls per load hold the
    PE clock (HAM) at 2.4 GHz through DMA-paced stretches.

hs and W_pred are bf16 on-chip (gather is exact; psum accumulates f32) and
the output travels bf16 over DMA before the host casts to f32, giving
~3e-3 relative error vs the f32 reference.
"""

import sys

import numpy as np

L, B, Q1, D = 6, 256, 101, 256
M, R, P = 64, 64, 51
NCORES = 8
BLOC = B // NCORES          # images per core
NB = L * BLOC               # (layer, image) blocks per core
G = 8                       # blocks per group (one 512-col psum region)
NG = NB // G                # groups per core
GCOLS = 4 * 512             # tile cols per group (4 chunks x 512)
SPLITS = [2, 2, 4, 4, 4, 4, 2, 2]   # groups per DMA load
NPAIR = NG // 2

_CACHE = {}


def _build_program():
    import concourse.bacc as bacc
    import concourse.mybir as mybir
    import concourse.tile as tile
    from contextlib import ExitStack

    f32 = mybir.dt.float32
    bf16 = mybir.dt.bfloat16
    nc = bacc.Bacc("TRN2", target_bir_lowering=False, debug=False)

    pg = nc.dram_tensor("pg", [128, NG * GCOLS], bf16,
                        kind="ExternalInput").ap()
    wt = nc.dram_tensor("wt", [128, 4 * P], bf16, kind="ExternalInput").ap()
    bias = nc.dram_tensor("bias", [128, 1], f32, kind="ExternalInput").ap()
    out = nc.dram_tensor("out", [NPAIR, 128, 512], bf16,
                         kind="ExternalOutput").ap()

    with tile.TileContext(nc) as tc, ExitStack() as ctx:
        const = ctx.enter_context(tc.tile_pool(name="const", bufs=1))
        inp = ctx.enter_context(tc.tile_pool(name="inp", bufs=3))
        outp = ctx.enter_context(tc.tile_pool(name="outp", bufs=4))
        psW = ctx.enter_context(tc.tile_pool(name="psW", bufs=1, space="PSUM"))
        psO = ctx.enter_context(tc.tile_pool(name="psO", bufs=6, space="PSUM"))

        wt_t = const.tile([128, 4 * P], bf16)
        nc.sync.dma_start(out=wt_t[:], in_=wt[:])
        bias_t = const.tile([128, 1], f32)
        nc.sync.dma_start(out=bias_t[:], in_=bias[:])

        # HAM warm-up: dense N=512 matmuls push the PE clock 1.2 -> 2.4 GHz
        wu = const.tile([128, 512], bf16)
        nc.vector.memset(wu[:], 0.0)
        wps = psW.tile([128, 512], f32, tag="wu")
        for _ in range(12):
            nc.tensor.matmul(out=wps[:], lhsT=wu[:, 0:128], rhs=wu[:],
                             start=True, stop=True)

        goff = 0
        pair_idx = 0
        pending = []          # (tile, col offset) per loaded group, in order
        for sz in SPLITS:
            pg_t = inp.tile([128, 4 * GCOLS], bf16, tag="pg")
            nc.gpsimd.dma_start(out=pg_t[:, 0:sz * GCOLS],
                                in_=pg[:, goff * GCOLS:(goff + sz) * GCOLS])
            pending.extend((pg_t, s * GCOLS) for s in range(sz))
            goff += sz
            # keep-warm matmuls run while the PE waits on this load
            for _ in range(sz // 2):
                nc.tensor.matmul(out=wps[:], lhsT=wu[:, 0:128], rhs=wu[:],
                                 start=True, stop=True)
            while len(pending) >= 2:
                (t0, o0), (t1, o1) = pending[0], pending[1]
                del pending[:2]
                pO = psO.tile([128, 512], f32, tag="pO")
                o_t = outp.tile([128, 512], bf16, tag="o")
                for e, (te, oe) in enumerate(((t0, o0), (t1, o1))):
                    rows = slice(64 * e, 64 * e + P)
                    for c in range(4):
                        nc.tensor.matmul(out=pO[rows, :],
                                         lhsT=wt_t[:, c * P:(c + 1) * P],
                                         rhs=te[:, oe + c * 512:
                                                oe + (c + 1) * 512],
                                         start=(c == 0), stop=(c == 3),
                                         tile_position=(0, 64 * e))
                    if e == 0:
                        nc.vector.tensor_scalar_add(out=o_t[rows, :],
                                                    in0=pO[rows, :],
                                                    scalar1=bias_t[rows, :])
                    else:
                        nc.scalar.add(out=o_t[rows, :], in_=pO[rows, :],
                                      add=bias_t[rows, :])
                qeng = (nc.scalar, nc.sync)[pair_idx % 2]
                qeng.dma_start(out=out[pair_idx], in_=o_t[:])
                pair_idx += 1

    nc.compile()
    return nc


def _host_indices(src_indices, tgt_perm, relationships):
    """q_sub, q_obj: [L, B, R] int64 — matched query slot per relation."""
    src = np.asarray(src_indices, dtype=np.int64)
    tgt = np.asarray(tgt_perm, dtype=np.int64)
    rel = np.asarray(relationships, dtype=np.int64)

    # lookup[l, b, tgt[l, b, k]] = k
    lookup = np.empty((L, B, M), dtype=np.int64)
    li = np.arange(L)[:, None, None]
    bi = np.arange(B)[None, :, None]
    lookup[li, bi, tgt] = np.broadcast_to(np.arange(M), (L, B, M))

    sub_t = np.broadcast_to(rel[None, :, :, 0], (L, B, R))
    obj_t = np.broadcast_to(rel[None, :, :, 1], (L, B, R))
    pos_sub = np.take_along_axis(lookup, sub_t, axis=2)
    pos_obj = np.take_along_axis(lookup, obj_t, axis=2)
    q_sub = np.take_along_axis(src, pos_sub, axis=2)
    q_obj = np.take_along_axis(src, pos_obj, axis=2)
    return q_sub, q_obj


def _host_prepare(hs, src_indices, tgt_perm, relationships, W_pred, b_pred):
    """Build per-core input maps."""
    import ml_dtypes
    bf16 = ml_dtypes.bfloat16

    hs_bf = np.asarray(hs, dtype=np.float32).astype(bf16)
    W = np.asarray(W_pred, dtype=np.float32)
    b = np.asarray(b_pred, dtype=np.float32)

    q_sub, q_obj = _host_indices(src_indices, tgt_perm, relationships)
    q_cat = np.concatenate([q_sub, q_obj], axis=-1)          # [L, B, 2R]
    # gathered[l, b, j, :] = hs[l, b, q_cat[l, b, j], :]
    gathered = np.take_along_axis(hs_bf, q_cat[..., None], axis=2)

    # W chunks: wt[:, c*P + p] = W[p, c*128 + dd]
    wt_packed = np.ascontiguousarray(
        W.reshape(P, 4, 128).transpose(2, 1, 0).reshape(128, 4 * P)
    ).astype(bf16)
    # bias at partitions 0:51 (even groups) and 64:115 (odd groups)
    bias_col = np.zeros((128, 1), dtype=np.float32)
    bias_col[0:P, 0] = b
    bias_col[64:64 + P, 0] = b

    in_maps = []
    for core in range(NCORES):
        sl = slice(core * BLOC, (core + 1) * BLOC)
        # cols must be (group, c=(so, dh), j, r), d on partitions
        arr = gathered[:, sl].reshape(NB, 2, R, 2, 128)   # [nb, so, r, dh, dd]
        arr = arr.reshape(NG, G, 2, R, 2, 128)            # [g, j, so, r, dh,dd]
        pg_core = np.ascontiguousarray(
            arr.transpose(5, 0, 2, 4, 1, 3))              # [dd, g, so, dh, j,r]
        in_maps.append({
            "pg": pg_core.reshape(128, NG * GCOLS),
            "wt": wt_packed,
            "bias": bias_col,
        })
    return in_maps


def kernel(hs, src_indices, tgt_perm, relationships, W_pred, b_pred):
    if "concourse" not in sys.modules:
        try:
            import concourse  # noqa: F401
        except ImportError:
            sys.path.insert(0, "/opt/trn_rl_repo")
    from concourse import bass_utils

    in_maps = _host_prepare(hs, src_indices, tgt_perm, relationships,
                            W_pred, b_pred)
    if "nc" not in _CACHE:
        _CACHE["nc"] = _build_program()
    nc = _CACHE["nc"]

    res = bass_utils.run_bass_kernel_spmd(nc, in_maps, list(range(NCORES)))
    outs = []
    for core in range(NCORES):
        o = res.results[core]["out"]              # [NPAIR, 128, 512] bf16
        o = np.asarray(o, dtype=np.float32)
        t = o.reshape(NPAIR, 128, G, R)           # [pairi, row, j, r]
        comb = np.stack([t[:, 0:P], t[:, 64:64 + P]], axis=1)
        # comb: [pairi, e, p, j, r] -> [pairi, e, j, r, p]
        arr = comb.transpose(0, 1, 3, 4, 2).reshape(NB, R, P)
        outs.append(arr.reshape(L, BLOC, R, P))
    return np.concatenate(outs, axis=1)
